# revision 28
# baseline (speedup 1.0000x reference)
"""AssociativeAttention kernel for 8 TRN2 NeuronCores.

Sharding strategy (per sharding_hint): heads are data-parallel - core i
owns head i (H=8 heads, 8 cores). Weights/filters are replicated. The
dominant FLOPs - the causal spectral convolution (24 filters x
block-Toeplitz [128,128] matmuls over k and v) - run on the
TensorEngine in bf16 (4x faster than fp32 matmul, fp32 PSUM
accumulation). The cheap surrounding elementwise/scan work stays on
host, vectorized across heads.

Self-contained: shapes hardcoded for B=1, L=1024, D=512, H=8, h=64, K=24.
"""

import os
import numpy as np

B, L, D, H, K = 1, 1024, 512, 8, 24
hd = D // H  # 64
EPS = 1e-5
NFFT = 2 * L

LAST_EXEC_NS = 0
_CACHE = {}


# ---------------------------------------------------------------------------
# Device graph: per core, compute causal conv of both kn and vn with all 24
# filters. Layout:
#   u   [128, 1024] bf16 : u[b, j*128 + t*64 + d] = (kn,vn)[t][j*128+b, d]
#   tz  [K, 128, 1024] bf16 : tz[kk][b, dlt*128+a] = f[dlt*128 + a - b, kk]
#   out [K, 128, 1024] bf16 : out[kk][a, j'*128 + t*64 + d]
#       = conv_kk((kn,vn)[t])[j'*128 + a, d]
# Per filter: PSUM bank A accumulates output blocks j'=0..3, bank B blocks
# j'=4..7, over diagonal offsets dlt (block-Toeplitz structure); k and v
# share the stationary weights via the interleaved u layout.
# ---------------------------------------------------------------------------

def _build_graph():
    import concourse.bass as bass
    import concourse.mybir as mybir
    from concourse.ap import AP
    from concourse.tile import TileContext
    from concourse.tile_rust import add_dep_helper

    f32 = mybir.dt.float32
    bf16 = mybir.dt.bfloat16
    nc = bass.Bass(target_bir_lowering=False)
    u_ext = nc.declare_dram_parameter("u", [128, 1024], bf16, isOutput=False)
    # fz[kk, 127 + m] = filters[m, kk], zero-padded front. The Toeplitz tile
    # for filter kk, REVERSED along the contraction index b, is the
    # overlapping stride-1 window t_rev[b, x] = fz[kk, b + x]; u is packed
    # b-reversed on host so lhsT.T @ rhs is unchanged.
    fz_ext = nc.declare_dram_parameter("fz", [K, 1152], bf16, isOutput=False)
    # One DRAM output per 3-filter group: distinct tensors avoid the WAW
    # sem-wait chain on a shared output (DIRECT2D DMA allows only one wait).
    outs_ext = [nc.declare_dram_parameter(f"out{g}", [128, 8 * 1024], bf16,
                                          isOutput=True)
                for g in range(K // 8)]
    fz_t = fz_ext[:, :].tensor

    with TileContext(nc) as tc:
        with (
            tc.tile_pool(name="upool", bufs=1) as up,
            tc.tile_pool(name="tstpool", bufs=4) as tsp,
            tc.tile_pool(name="tzpool", bufs=24) as tzp,
            tc.tile_pool(name="opool", bufs=3) as op_,
            tc.tile_pool(name="pspool", bufs=4, space="PSUM") as pp,
        ):
            u = up.tile([128, 1024], bf16)
            pre_drain = [nc.sync.dma_start(out=u[:, :], in_=u_ext[:, :])]
            og_release = []
            ts6 = None
            for kk in range(K):
                # DMA -> staging, DVE copy -> t: the matmuls' weight tile is
                # DVE-written, so the Ldweights' DVE wait subsumes the PSUM
                # slot-release wait and each bf16 Matmult keeps <=1 sem wait
                # (the S3D3_MM walrus codegen limit).
                if kk % 6 == 0:
                    ts6 = tsp.tile([128, 6 * 1024], bf16, tag="tst")
                    win = AP(tensor=fz_t, offset=kk * 1152,
                             ap=[[1, 128], [1152, 6], [1, 1024]])
                    pre_drain.append(
                        nc.sync.dma_start(out=ts6[:, :], in_=win))
                soff = (kk % 6) * 1024
                t = tzp.tile([128, 1024], bf16, tag="tz")
                tci = nc.scalar.copy(t[:, :], ts6[:, soff:soff + 1024])
                if kk >= 4:
                    # Keep DVE order: t-copy(kk) after the PSUM-release
                    # copies of filter kk-4, so the Ldweights' single DVE
                    # wait subsumes the start-matmul's bank-release wait
                    # (every instruction gets at most one sem wait).
                    add_dep_helper(tci.ins, og_release[kk - 4].ins,
                                   sync=False,
                                   reason="order t-copy after psum release")
                pz = pp.tile([128, 1024], f32, tag="pz")
                pa = pz[:, 0:512]
                pb = pz[:, 512:1024]
                for dlt in range(4):
                    nA = (4 - dlt) * 128
                    nc.tensor.matmul(pa[:, dlt * 128:512],
                                     lhsT=t[:, dlt * 128:(dlt + 1) * 128],
                                     rhs=u[:, 0:nA],
                                     start=(dlt == 0), stop=(dlt == 3))
                for dlt in range(8):
                    jm = max(0, 4 - dlt)
                    nB = min(4, 8 - dlt) * 128
                    last_mm = nc.tensor.matmul(
                        pb[:, (max(dlt, 4) - 4) * 128:512],
                        lhsT=t[:, dlt * 128:(dlt + 1) * 128],
                        rhs=u[:, jm * 128:jm * 128 + nB],
                        start=(dlt == 0), stop=(dlt == 7))
                if kk % 8 == 0:
                    og = op_.tile([128, 8 * 1024], bf16, tag="og")
                off = (kk % 8) * 1024
                og_release.append(
                    nc.scalar.copy(og[:, off:off + 1024], pz[:, :]))
                if kk % 8 == 7:
                    pre_drain.append(
                        nc.scalar.dma_start(out=outs_ext[kk // 8][:, :],
                                            in_=og[:, :]))
            # Pre-drain wait ladder: one SP nop per outstanding proc, each
            # carrying a single sem wait, so the framework tail drain's
            # add_sem_waits finds the SP clock already caught up and emits
            # no multi-wait instruction (walrus limits sync waits per inst).
            for dep in pre_drain + [last_mm, og_release[-1]]:
                nop = nc.vector.engine_nop()
                add_dep_helper(nop.ins, dep.ins, sync=True,
                               reason="pre-drain wait ladder")
    # The SP nop ladder above already waited (in order, one sem each) on
    # every outstanding proc; the framework tail drain's aggregated waits
    # are therefore redundant on the in-order SP queue. Trim them to one so
    # no instruction exceeds the walrus single-sync-wait codegen limit.
    for b in nc.m.functions[0].blocks:
        for i in b.instructions:
            si = getattr(i, 'sync_info', None)
            if si is not None and len(si.on_wait) > 1:
                si2 = type(si)(on_wait=[si.on_wait[-1]],
                               on_update=si.on_update)
                i.sync_info = si2
    return nc


def _toeplitz_tiles(filters):
    """tz[k, b, dlt*128+a] = f[dlt*128 + a - b, k] (0 where index < 0)."""
    fpad = np.zeros((127 + L, K), np.float32)
    fpad[127:] = filters
    tz = np.empty((K, 128, 8 * 128), np.float32)
    a = np.arange(128)
    idx = 127 + a[None, :] - a[:, None]          # [b, a] base indices
    for dlt in range(8):
        blk = fpad[idx + dlt * 128]              # [b, a, K]
        tz[:, :, dlt * 128:(dlt + 1) * 128] = np.moveaxis(blk, 2, 0)
    return tz


def _device_conv(kn_all, vn_all, filters):
    """kn_all/vn_all: [H, L, hd] normalized k/v per head.
    Returns kc, vc: [H, L, K, hd] float32 via SPMD conv on 8 cores."""
    global LAST_EXEC_NS
    import ml_dtypes
    from concourse.bass_utils import run_bass_kernel_spmd

    if "nc" not in _CACHE:
        _CACHE["nc"] = _build_graph()
    nc = _CACHE["nc"]

    bf16 = ml_dtypes.bfloat16
    fz = np.zeros((K, 1152), np.float32)
    fz[:, 127:127 + L] = filters.T               # fz[kk, 127+m] = f[m, kk]
    fzb = fz.astype(bf16)
    # u[b, j*128 + t*64 + d] = (kn, vn)[t][j*128 + b, d], then b-reversed to
    # match the b-reversed Toeplitz windows read from fz on device.
    in_maps = []
    for head in range(H):
        um = np.empty((128, 1024), np.float32)
        stacked = np.stack((kn_all[head], vn_all[head]), axis=1)  # [L,2,hd]
        um[:] = (stacked.reshape(8, 128, 2 * hd)
                 .transpose(1, 0, 2).reshape(128, 1024))
        um_rev = np.ascontiguousarray(um[::-1])
        in_maps.append({"u": um_rev.astype(bf16), "fz": fzb})

    import time
    res = run_bass_kernel_spmd(nc, in_maps, core_ids=list(range(H)))
    if getattr(res, "exec_time_ns", None):
        LAST_EXEC_NS = res.exec_time_ns
    elif os.environ.get("BASS_TIME_2ND") == "1":
        # No NTFF profiling hook under this axon client: approximate the
        # device time by timing a warm re-execution (NEFF cached), which
        # upper-bounds exec time by the RPC overhead.
        t0 = time.perf_counter()
        run_bass_kernel_spmd(nc, in_maps, core_ids=list(range(H)))
        LAST_EXEC_NS = int((time.perf_counter() - t0) * 1e9)

    kc = np.empty((H, L, K, hd), np.float32)
    vc = np.empty((H, L, K, hd), np.float32)
    for head in range(H):
        o = np.stack([np.asarray(res.results[head][f"out{g}"],
                                 dtype=np.float32).reshape(128, 8, 1024)
                      for g in range(K // 8)], axis=1)  # [a, g, 8, 1024]
        o = o.reshape(128, K, 8, 2, hd)          # [a, kk, j', t, d]
        kc[head] = o[:, :, :, 0].transpose(2, 0, 1, 3).reshape(L, K, hd)
        vc[head] = o[:, :, :, 1].transpose(2, 0, 1, 3).reshape(L, K, hd)
    return kc, vc


def _device_impl(x, Wq, bq, Wk, bk, Wv, bv, Wo, bo, Wg, bg,
                 kv_norm_scale, qk_norm_scale, spectral_filters):
    xb = x[0]                                    # [L, D]
    q = (xb @ Wq + bq).reshape(L, H, hd).transpose(1, 0, 2)   # [H,L,hd]
    k = (xb @ Wk + bk).reshape(L, H, hd).transpose(1, 0, 2)
    v = (xb @ Wv + bv).reshape(L, H, hd).transpose(1, 0, 2)

    sim = (q * k).sum(-1) * qk_norm_scale[0, :, :]            # [H,L]
    kn = k / np.maximum(np.linalg.norm(k, axis=-1, keepdims=True), 1e-12)
    vn = v / np.maximum(np.linalg.norm(v, axis=-1, keepdims=True), 1e-12)

    kc, vc = _device_conv(kn, vn, spectral_filters)           # [H,L,K,hd]

    # Z[h,l,d,e] = sum_k vc[h,l,k,d] * kc[h,l,k,e], batched across (h,l)
    kvs = kv_norm_scale[0, :, 0]                              # [H,hd,hd]
    Z = np.matmul(vc.reshape(H * L, K, hd).transpose(0, 2, 1),
                  kc.reshape(H * L, K, hd)).reshape(H, L, hd, hd)
    Z *= kvs[:, None]

    logits = Z.reshape(H * L, hd * hd) @ Wg + bg              # [H*L,1]
    g = (np.maximum(logits[:, 0], 0.0) ** 2 + EPS).reshape(H, L)

    Z_scan = np.cumsum((g[:, :, None, None] * Z).astype(np.float64),
                       axis=1).astype(np.float32)
    g_scan = np.cumsum(g.astype(np.float64), axis=1).astype(np.float32)

    m_scan = np.maximum.accumulate(sim, axis=1)
    lse = np.logaddexp.accumulate(sim.astype(np.float64), axis=1)
    s_scan = np.exp(lse - m_scan).astype(np.float32)
    sw = np.exp(sim - m_scan) / (s_scan + EPS)
    coef = 1.0 + sw / (1.0 + np.exp(-sw))                     # [H,L]

    gw = Z_scan / (g_scan[:, :, None, None] + EPS)            # [H,L,hd,hd]
    ctxt = np.matmul(q.reshape(H * L, 1, hd),
                     gw.reshape(H * L, hd, hd))[:, 0]
    ctxt = (ctxt.reshape(H, L, hd) * coef[:, :, None])
    # out = sum_h ctxt_h @ Wo[h*hd:(h+1)*hd, :] + bo
    out = np.einsum('hld,hde->le', ctxt.astype(np.float64),
                    Wo.reshape(H, hd, D).astype(np.float64))
    return (out + bo).astype(np.float32)[None]


# ---------------------------------------------------------------------------
# Host fallback (exact, FFT-based) - used only if the device path fails.
# ---------------------------------------------------------------------------

def _conv_full(filters, u):
    """filters [L,K], u [H,L,h] -> causal FFT conv [H,L,K,h] (float32)."""
    Ff = np.fft.rfft(filters.astype(np.float64), n=NFFT, axis=0)   # [F,K]
    U = np.fft.rfft(u.astype(np.float64), n=NFFT, axis=1)          # [H,F,h]
    y = np.fft.irfft(U[:, :, None, :] * Ff[None, :, :, None],
                     n=NFFT, axis=1)                               # [H,NFFT,K,h]
    return y[:, :L].astype(np.float32)


def _host_impl(x, Wq, bq, Wk, bk, Wv, bv, Wo, bo, Wg, bg,
               kv_norm_scale, qk_norm_scale, spectral_filters):
    xb = x[0]
    q = (xb @ Wq + bq).reshape(L, H, hd).transpose(1, 0, 2)
    k = (xb @ Wk + bk).reshape(L, H, hd).transpose(1, 0, 2)
    v = (xb @ Wv + bv).reshape(L, H, hd).transpose(1, 0, 2)

    sim = (q * k).sum(-1) * qk_norm_scale[0, :, :]
    kn = k / np.maximum(np.linalg.norm(k, axis=-1, keepdims=True), 1e-12)
    vn = v / np.maximum(np.linalg.norm(v, axis=-1, keepdims=True), 1e-12)

    kc = _conv_full(spectral_filters, kn)
    vc = _conv_full(spectral_filters, vn)

    kvs = kv_norm_scale[0, :, 0]
    Z = np.matmul(vc.reshape(H * L, K, hd).transpose(0, 2, 1),
                  kc.reshape(H * L, K, hd)).reshape(H, L, hd, hd)
    Z *= kvs[:, None]

    logits = Z.reshape(H * L, hd * hd) @ Wg + bg
    g = (np.maximum(logits[:, 0], 0.0) ** 2 + EPS).reshape(H, L)

    Z_scan = np.cumsum((g[:, :, None, None] * Z).astype(np.float64),
                       axis=1).astype(np.float32)
    g_scan = np.cumsum(g.astype(np.float64), axis=1).astype(np.float32)

    m_scan = np.maximum.accumulate(sim, axis=1)
    lse = np.logaddexp.accumulate(sim.astype(np.float64), axis=1)
    s_scan = np.exp(lse - m_scan).astype(np.float32)
    sw = np.exp(sim - m_scan) / (s_scan + EPS)
    coef = 1.0 + sw / (1.0 + np.exp(-sw))

    gw = Z_scan / (g_scan[:, :, None, None] + EPS)
    ctxt = np.matmul(q.reshape(H * L, 1, hd),
                     gw.reshape(H * L, hd, hd))[:, 0]
    ctxt = (ctxt.reshape(H, L, hd) * coef[:, :, None])
    out = np.einsum('hld,hde->le', ctxt.astype(np.float64),
                    Wo.reshape(H, hd, D).astype(np.float64))
    return (out + bo).astype(np.float32)[None]


def kernel(**inputs):
    inputs = {k_: np.ascontiguousarray(np.asarray(v, dtype=np.float32))
              for k_, v in inputs.items()}
    try:
        return _device_impl(**inputs)
    except Exception:
        return _host_impl(**inputs)


if __name__ == '__main__':
    pass


# revision 32
# speedup vs baseline: 1.0521x; 1.0521x over previous
"""AssociativeAttention kernel for 8 TRN2 NeuronCores.

Sharding strategy (per sharding_hint): heads are data-parallel - core i
owns head i (H=8 heads, 8 cores). Weights/filters are replicated. The
dominant FLOPs - the causal spectral convolution (24 filters x
block-Toeplitz [128,128] matmuls over k and v) - run on the
TensorEngine in bf16 (4x faster than fp32 matmul, fp32 PSUM
accumulation). The cheap surrounding elementwise/scan work stays on
host, vectorized across heads.

Self-contained: shapes hardcoded for B=1, L=1024, D=512, H=8, h=64, K=24.
"""

import os
import numpy as np

B, L, D, H, K = 1, 1024, 512, 8, 24
hd = D // H  # 64
EPS = 1e-5
NFFT = 2 * L

LAST_EXEC_NS = 0
_CACHE = {}


# ---------------------------------------------------------------------------
# Device graph: per core, compute causal conv of both kn and vn with all 24
# filters. Layout:
#   u   [128, 1024] bf16 : u[b, j*128 + t*64 + d] = (kn,vn)[t][j*128+b, d]
#   tz  [K, 128, 1024] bf16 : tz[kk][b, dlt*128+a] = f[dlt*128 + a - b, kk]
#   out [K, 128, 1024] bf16 : out[kk][a, j'*128 + t*64 + d]
#       = conv_kk((kn,vn)[t])[j'*128 + a, d]
# Per filter: PSUM bank A accumulates output blocks j'=0..3, bank B blocks
# j'=4..7, over diagonal offsets dlt (block-Toeplitz structure); k and v
# share the stationary weights via the interleaved u layout.
# ---------------------------------------------------------------------------

def _build_graph():
    import concourse.bass as bass
    import concourse.mybir as mybir
    from concourse.ap import AP
    from concourse.tile import TileContext
    from concourse.tile_rust import add_dep_helper

    f32 = mybir.dt.float32
    bf16 = mybir.dt.bfloat16
    nc = bass.Bass(target_bir_lowering=False)
    u_ext = nc.declare_dram_parameter("u", [128, 1024], bf16, isOutput=False)
    # fz[kk, 127 + m] = filters[m, kk], zero-padded front. The Toeplitz tile
    # for filter kk, REVERSED along the contraction index b, is the
    # overlapping stride-1 window t_rev[b, x] = fz[kk, b + x]; u is packed
    # b-reversed on host so lhsT.T @ rhs is unchanged.
    fz_ext = nc.declare_dram_parameter("fz", [K, 1152], bf16, isOutput=False)
    # One DRAM output per 3-filter group: distinct tensors avoid the WAW
    # sem-wait chain on a shared output (DIRECT2D DMA allows only one wait).
    outs_ext = [nc.declare_dram_parameter(f"out{g}", [128, 8 * 1024], bf16,
                                          isOutput=True)
                for g in range(K // 8)]
    fz_t = fz_ext[:, :].tensor

    with TileContext(nc) as tc:
        with (
            tc.tile_pool(name="upool", bufs=1) as up,
            tc.tile_pool(name="tstpool", bufs=4) as tsp,
            tc.tile_pool(name="tzpool", bufs=24) as tzp,
            tc.tile_pool(name="opool", bufs=3) as op_,
            tc.tile_pool(name="pspool", bufs=4, space="PSUM") as pp,
        ):
            u = up.tile([128, 1024], bf16)
            pre_drain = [nc.sync.dma_start(out=u[:, :], in_=u_ext[:, :])]
            og_release = []
            ts6 = None
            for kk in range(K):
                # DMA -> staging, DVE copy -> t: the matmuls' weight tile is
                # DVE-written, so the Ldweights' DVE wait subsumes the PSUM
                # slot-release wait and each bf16 Matmult keeps <=1 sem wait
                # (the S3D3_MM walrus codegen limit).
                if kk % 6 == 0:
                    ts6 = tsp.tile([128, 6 * 1024], bf16, tag="tst")
                    win = AP(tensor=fz_t, offset=kk * 1152,
                             ap=[[1, 128], [1152, 6], [1, 1024]])
                    pre_drain.append(
                        nc.sync.dma_start(out=ts6[:, :], in_=win))
                soff = (kk % 6) * 1024
                t = tzp.tile([128, 1024], bf16, tag="tz")
                tci = nc.scalar.copy(t[:, :], ts6[:, soff:soff + 1024])
                if kk >= 4:
                    # Keep DVE order: t-copy(kk) after the PSUM-release
                    # copies of filter kk-4, so the Ldweights' single DVE
                    # wait subsumes the start-matmul's bank-release wait
                    # (every instruction gets at most one sem wait).
                    add_dep_helper(tci.ins, og_release[kk - 4].ins,
                                   sync=False,
                                   reason="order t-copy after psum release")
                pz = pp.tile([128, 1024], f32, tag="pz")
                pa = pz[:, 0:512]
                pb = pz[:, 512:1024]
                for dlt in range(4):
                    nA = (4 - dlt) * 128
                    nc.tensor.matmul(pa[:, dlt * 128:512],
                                     lhsT=t[:, dlt * 128:(dlt + 1) * 128],
                                     rhs=u[:, 0:nA],
                                     start=(dlt == 0), stop=(dlt == 3))
                for dlt in range(8):
                    jm = max(0, 4 - dlt)
                    nB = min(4, 8 - dlt) * 128
                    last_mm = nc.tensor.matmul(
                        pb[:, (max(dlt, 4) - 4) * 128:512],
                        lhsT=t[:, dlt * 128:(dlt + 1) * 128],
                        rhs=u[:, jm * 128:jm * 128 + nB],
                        start=(dlt == 0), stop=(dlt == 7))
                if kk % 8 == 0:
                    og = op_.tile([128, 8 * 1024], bf16, tag="og")
                off = (kk % 8) * 1024
                og_release.append(
                    nc.scalar.copy(og[:, off:off + 1024], pz[:, :]))
                if kk % 8 == 7:
                    pre_drain.append(
                        nc.scalar.dma_start(out=outs_ext[kk // 8][:, :],
                                            in_=og[:, :]))
            # Pre-drain wait ladder: one SP nop per outstanding proc, each
            # carrying a single sem wait, so the framework tail drain's
            # add_sem_waits finds the SP clock already caught up and emits
            # no multi-wait instruction (walrus limits sync waits per inst).
            for dep in pre_drain + [last_mm, og_release[-1]]:
                nop = nc.vector.engine_nop()
                add_dep_helper(nop.ins, dep.ins, sync=True,
                               reason="pre-drain wait ladder")
    # The DVE nop ladder above already waited (in order, one sem each) on
    # every outstanding proc, so the framework tail Drain's aggregated waits
    # are redundant; trim them to one so no instruction exceeds the walrus
    # single-sync-wait codegen limit. ONLY Drains may be trimmed - for any
    # other instruction a dropped wait is a real race (verified by CoreSim's
    # race detector), so assert instead.
    for b in nc.m.functions[0].blocks:
        for i in b.instructions:
            si = getattr(i, 'sync_info', None)
            if si is None or len(si.on_wait) <= 1:
                continue
            assert type(i).__name__ == 'InstDrain', (
                f"{i.name} ({type(i).__name__}) carries "
                f"{len(si.on_wait)} sem waits; restructure the graph "
                f"instead of trimming (unsafe)")
            i.sync_info = type(si)(on_wait=[si.on_wait[-1]],
                                   on_update=si.on_update)
    return nc


def _toeplitz_tiles(filters):
    """tz[k, b, dlt*128+a] = f[dlt*128 + a - b, k] (0 where index < 0)."""
    fpad = np.zeros((127 + L, K), np.float32)
    fpad[127:] = filters
    tz = np.empty((K, 128, 8 * 128), np.float32)
    a = np.arange(128)
    idx = 127 + a[None, :] - a[:, None]          # [b, a] base indices
    for dlt in range(8):
        blk = fpad[idx + dlt * 128]              # [b, a, K]
        tz[:, :, dlt * 128:(dlt + 1) * 128] = np.moveaxis(blk, 2, 0)
    return tz


def _device_conv(kn_all, vn_all, filters):
    """kn_all/vn_all: [H, L, hd] normalized k/v per head.
    Returns kc, vc: [H, L, K, hd] float32 via SPMD conv on 8 cores."""
    global LAST_EXEC_NS
    import ml_dtypes
    from concourse.bass_utils import run_bass_kernel_spmd

    if "nc" not in _CACHE:
        _CACHE["nc"] = _build_graph()
    nc = _CACHE["nc"]

    bf16 = ml_dtypes.bfloat16
    fz = np.zeros((K, 1152), np.float32)
    fz[:, 127:127 + L] = filters.T               # fz[kk, 127+m] = f[m, kk]
    fzb = fz.astype(bf16)
    # u[b, j*128 + t*64 + d] = (kn, vn)[t][j*128 + b, d], then b-reversed to
    # match the b-reversed Toeplitz windows read from fz on device.
    in_maps = []
    for head in range(H):
        um = np.empty((128, 1024), np.float32)
        stacked = np.stack((kn_all[head], vn_all[head]), axis=1)  # [L,2,hd]
        um[:] = (stacked.reshape(8, 128, 2 * hd)
                 .transpose(1, 0, 2).reshape(128, 1024))
        um_rev = np.ascontiguousarray(um[::-1])
        in_maps.append({"u": um_rev.astype(bf16), "fz": fzb})

    import time
    res = run_bass_kernel_spmd(nc, in_maps, core_ids=list(range(H)))
    if getattr(res, "exec_time_ns", None):
        LAST_EXEC_NS = res.exec_time_ns
    elif os.environ.get("BASS_TIME_2ND") == "1":
        # No NTFF profiling hook under this axon client: approximate the
        # device time by timing a warm re-execution (NEFF cached), which
        # upper-bounds exec time by the RPC overhead.
        t0 = time.perf_counter()
        run_bass_kernel_spmd(nc, in_maps, core_ids=list(range(H)))
        LAST_EXEC_NS = int((time.perf_counter() - t0) * 1e9)

    kc = np.empty((H, L, K, hd), np.float32)
    vc = np.empty((H, L, K, hd), np.float32)
    for head in range(H):
        o = np.stack([np.asarray(res.results[head][f"out{g}"],
                                 dtype=np.float32).reshape(128, 8, 1024)
                      for g in range(K // 8)], axis=1)  # [a, g, 8, 1024]
        o = o.reshape(128, K, 8, 2, hd)          # [a, kk, j', t, d]
        kc[head] = o[:, :, :, 0].transpose(2, 0, 1, 3).reshape(L, K, hd)
        vc[head] = o[:, :, :, 1].transpose(2, 0, 1, 3).reshape(L, K, hd)
    return kc, vc


def _device_impl(x, Wq, bq, Wk, bk, Wv, bv, Wo, bo, Wg, bg,
                 kv_norm_scale, qk_norm_scale, spectral_filters):
    xb = x[0]                                    # [L, D]
    q = (xb @ Wq + bq).reshape(L, H, hd).transpose(1, 0, 2)   # [H,L,hd]
    k = (xb @ Wk + bk).reshape(L, H, hd).transpose(1, 0, 2)
    v = (xb @ Wv + bv).reshape(L, H, hd).transpose(1, 0, 2)

    sim = (q * k).sum(-1) * qk_norm_scale[0, :, :]            # [H,L]
    kn = k / np.maximum(np.linalg.norm(k, axis=-1, keepdims=True), 1e-12)
    vn = v / np.maximum(np.linalg.norm(v, axis=-1, keepdims=True), 1e-12)

    kc, vc = _device_conv(kn, vn, spectral_filters)           # [H,L,K,hd]

    # Z[h,l,d,e] = sum_k vc[h,l,k,d] * kc[h,l,k,e], batched across (h,l)
    kvs = kv_norm_scale[0, :, 0]                              # [H,hd,hd]
    Z = np.matmul(vc.reshape(H * L, K, hd).transpose(0, 2, 1),
                  kc.reshape(H * L, K, hd)).reshape(H, L, hd, hd)
    Z *= kvs[:, None]

    logits = Z.reshape(H * L, hd * hd) @ Wg + bg              # [H*L,1]
    g = (np.maximum(logits[:, 0], 0.0) ** 2 + EPS).reshape(H, L)

    Z_scan = np.cumsum((g[:, :, None, None] * Z).astype(np.float64),
                       axis=1).astype(np.float32)
    g_scan = np.cumsum(g.astype(np.float64), axis=1).astype(np.float32)

    m_scan = np.maximum.accumulate(sim, axis=1)
    lse = np.logaddexp.accumulate(sim.astype(np.float64), axis=1)
    s_scan = np.exp(lse - m_scan).astype(np.float32)
    sw = np.exp(sim - m_scan) / (s_scan + EPS)
    coef = 1.0 + sw / (1.0 + np.exp(-sw))                     # [H,L]

    gw = Z_scan / (g_scan[:, :, None, None] + EPS)            # [H,L,hd,hd]
    ctxt = np.matmul(q.reshape(H * L, 1, hd),
                     gw.reshape(H * L, hd, hd))[:, 0]
    ctxt = (ctxt.reshape(H, L, hd) * coef[:, :, None])
    # out = sum_h ctxt_h @ Wo[h*hd:(h+1)*hd, :] + bo
    out = np.einsum('hld,hde->le', ctxt.astype(np.float64),
                    Wo.reshape(H, hd, D).astype(np.float64))
    return (out + bo).astype(np.float32)[None]


# ---------------------------------------------------------------------------
# Host fallback (exact, FFT-based) - used only if the device path fails.
# ---------------------------------------------------------------------------

def _conv_full(filters, u):
    """filters [L,K], u [H,L,h] -> causal FFT conv [H,L,K,h] (float32)."""
    Ff = np.fft.rfft(filters.astype(np.float64), n=NFFT, axis=0)   # [F,K]
    U = np.fft.rfft(u.astype(np.float64), n=NFFT, axis=1)          # [H,F,h]
    y = np.fft.irfft(U[:, :, None, :] * Ff[None, :, :, None],
                     n=NFFT, axis=1)                               # [H,NFFT,K,h]
    return y[:, :L].astype(np.float32)


def _host_impl(x, Wq, bq, Wk, bk, Wv, bv, Wo, bo, Wg, bg,
               kv_norm_scale, qk_norm_scale, spectral_filters):
    xb = x[0]
    q = (xb @ Wq + bq).reshape(L, H, hd).transpose(1, 0, 2)
    k = (xb @ Wk + bk).reshape(L, H, hd).transpose(1, 0, 2)
    v = (xb @ Wv + bv).reshape(L, H, hd).transpose(1, 0, 2)

    sim = (q * k).sum(-1) * qk_norm_scale[0, :, :]
    kn = k / np.maximum(np.linalg.norm(k, axis=-1, keepdims=True), 1e-12)
    vn = v / np.maximum(np.linalg.norm(v, axis=-1, keepdims=True), 1e-12)

    kc = _conv_full(spectral_filters, kn)
    vc = _conv_full(spectral_filters, vn)

    kvs = kv_norm_scale[0, :, 0]
    Z = np.matmul(vc.reshape(H * L, K, hd).transpose(0, 2, 1),
                  kc.reshape(H * L, K, hd)).reshape(H, L, hd, hd)
    Z *= kvs[:, None]

    logits = Z.reshape(H * L, hd * hd) @ Wg + bg
    g = (np.maximum(logits[:, 0], 0.0) ** 2 + EPS).reshape(H, L)

    Z_scan = np.cumsum((g[:, :, None, None] * Z).astype(np.float64),
                       axis=1).astype(np.float32)
    g_scan = np.cumsum(g.astype(np.float64), axis=1).astype(np.float32)

    m_scan = np.maximum.accumulate(sim, axis=1)
    lse = np.logaddexp.accumulate(sim.astype(np.float64), axis=1)
    s_scan = np.exp(lse - m_scan).astype(np.float32)
    sw = np.exp(sim - m_scan) / (s_scan + EPS)
    coef = 1.0 + sw / (1.0 + np.exp(-sw))

    gw = Z_scan / (g_scan[:, :, None, None] + EPS)
    ctxt = np.matmul(q.reshape(H * L, 1, hd),
                     gw.reshape(H * L, hd, hd))[:, 0]
    ctxt = (ctxt.reshape(H, L, hd) * coef[:, :, None])
    out = np.einsum('hld,hde->le', ctxt.astype(np.float64),
                    Wo.reshape(H, hd, D).astype(np.float64))
    return (out + bo).astype(np.float32)[None]


def kernel(**inputs):
    inputs = {k_: np.ascontiguousarray(np.asarray(v, dtype=np.float32))
              for k_, v in inputs.items()}
    try:
        return _device_impl(**inputs)
    except Exception:
        return _host_impl(**inputs)


if __name__ == '__main__':
    pass
